# revision 11
# baseline (speedup 1.0000x reference)
"""BP-MLL loss kernel for Trainium2 (8 NeuronCores, data-parallel over batch).

Math: for each sample b with scores o and binary labels y,
  pair_sums[b] = sum_{i in pos, j in neg} exp(o_j - o_i)
               = (sum_{j in neg} exp(o_j)) * (sum_{i in pos} exp(-o_i))
  y_norm[b]    = n_pos * (C - n_pos)
  loss         = sum_b pair_sums[b] / y_norm[b] / B

Each core gets 4 samples. The host packs them into one [128, 128] f32
buffer per core (cols 0:64 = scores, cols 64:128 = labels as f32; sample
b owns partitions 32b..32b+31), so the device does a single input DMA
and no int conversion. The device emits a [128, 4] per-partition stats
tile:
  col 0: sum exp(x)      col 1: sum y*exp(x)
  col 2: sum y*exp(-x)   col 3: sum y
The host finishes the tiny segmented reduction (neg-sum = col0 - col1).
"""

import sys

for _p in ("/opt/trn_rl_repo", "/root/.axon_site/_ro/trn_rl_repo"):
    if _p not in sys.path:
        sys.path.insert(0, _p)

import numpy as np

import concourse.bass as bass
import concourse.mybir as mybir
from concourse.bass_utils import run_bass_kernel_spmd

B, C = 32, 2048
N_CORES = 8
BPC = B // N_CORES            # samples per core (4)
P = 128                       # SBUF partitions
F = BPC * C // P              # free elems per partition (64)
PPS = P // BPC                # partitions per sample (32)

_NC_CACHE = {}
# Extra kwargs for run_bass_kernel_spmd (e.g. trace=True from a test harness).
_RUN_KWARGS = {}


def _build_bass(sim_safe=False):
    # sim_safe adds same-engine sem hops that CoreSim's race detector wants
    # for back-to-back DVE RAW; hardware executes each engine in order, so
    # the shipped program omits them.
    nc = bass.Bass(
        "TRN2", enable_partition_id=False, detect_race_conditions=sim_safe
    )
    fp32 = mybir.dt.float32
    x_d = nc.declare_dram_parameter("x", [P, 2 * F], fp32, isOutput=False)
    o_d = nc.declare_dram_parameter("out", [P, 4], fp32, isOutput=True)

    with (
        nc.sbuf_tensor([P, 2 * F], fp32) as xt,
        nc.sbuf_tensor([P, F], fp32) as ep,
        nc.sbuf_tensor([P, F], fp32) as em,
        nc.sbuf_tensor([P, F], fp32) as scratch0,
        nc.sbuf_tensor([P, F], fp32) as scratch1,
        nc.sbuf_tensor([P, 1], fp32) as warm,
        nc.sbuf_tensor([P, 4], fp32) as ot,
        nc.semaphore("dsem") as dsem,
        nc.semaphore("esem") as esem,
        nc.semaphore("vsem") as vsem,
        nc.semaphore("ssem") as ssem,
        nc.Block(no_gpsimd_drain=True) as block,
    ):
        xs = xt[:, 0:F]       # scores
        ys = xt[:, F : 2 * F]  # labels (0.0/1.0)

        @block.sync
        def _(sync):
            sync.dma_start(out=xt[:], in_=x_d[:]).then_inc(dsem, 16)
            sync.wait_ge(vsem, 1)
            sync.dma_start(out=o_d[:], in_=ot[:]).then_inc(dsem, 16)

        @block.scalar
        def _(scalar):
            # Warm the Exp activation table while the input DMA is in flight.
            zero = nc.const_aps.scalar_like(0.0, warm[:, 0:1])
            scalar.activation(warm[:, 0:1], zero, mybir.ActivationFunctionType.Exp)
            scalar.wait_ge(dsem, 16)
            scalar.activation(
                ep[:], xs, mybir.ActivationFunctionType.Exp
            ).then_inc(esem, 1)
            scalar.activation(
                em[:], xs, mybir.ActivationFunctionType.Exp, scale=-1.0
            ).then_inc(esem, 1)

        @block.vector
        def _(vector):
            X = mybir.AxisListType.X
            vector.wait_ge(dsem, 16)
            vector.reduce_sum(ot[:, 3:4], ys, axis=X)          # col 3: n_pos
            vector.wait_ge(esem, 1)
            vector.reduce_sum(ot[:, 0:1], ep[:], axis=X)       # col 0: sum exp(x)
            m0 = vector.tensor_mul(scratch0[:], ep[:], ys)
            if sim_safe:
                m0.then_inc(ssem, 1)
                vector.wait_ge(ssem, 1)
            vector.reduce_sum(ot[:, 1:2], scratch0[:], axis=X)  # col 1: sum y*exp(x)
            vector.wait_ge(esem, 2)
            m1 = vector.tensor_mul(scratch1[:], em[:], ys)
            if sim_safe:
                m1.then_inc(ssem, 1)
                vector.wait_ge(ssem, 2)
            vector.reduce_sum(
                ot[:, 2:3], scratch1[:], axis=X
            ).then_inc(vsem, 1)                                 # col 2: sum y*exp(-x)

    # Raw Bass skips Bacc's codegen_inst_isa_subclasses pass; without it any
    # extended-ISA instructions have empty .instr bytes and walrus codegen
    # fails with "ISA wrong length".
    mybir.codegen_inst_isa_subclasses(nc)
    return nc


def _get_nc():
    if "nc" not in _NC_CACHE:
        _NC_CACHE["nc"] = _build_bass()
    return _NC_CACHE["nc"]


def _pack(input, target):
    """Per-core [128, 128] f32: cols 0:64 scores, cols 64:128 labels."""
    maps = []
    for i in range(N_CORES):
        sl = slice(i * BPC, (i + 1) * BPC)
        buf = np.empty((P, 2 * F), dtype=np.float32)
        buf[:, :F] = input[sl].reshape(P, F)
        buf[:, F:] = target[sl].astype(np.float32).reshape(P, F)
        maps.append({"x": buf})
    return maps


def kernel(input, target, _results_out=None):
    input = np.ascontiguousarray(np.asarray(input, dtype=np.float32))
    target = np.ascontiguousarray(np.asarray(target, dtype=np.int32))
    assert input.shape == (B, C) and target.shape == (B, C)

    nc = _get_nc()
    in_maps = _pack(input, target)
    res = run_bass_kernel_spmd(nc, in_maps, core_ids=list(range(N_CORES)), **_RUN_KWARGS)
    if _results_out is not None:
        _results_out.append(res)

    total = np.float32(0.0)
    for i in range(N_CORES):
        stats = res.results[i]["out"]            # [128, 4] f32
        per_sample = stats.reshape(BPC, PPS, 4).sum(axis=1, dtype=np.float32)
        s_all, s_pos, s_posinv, cnt = per_sample.T
        s_neg = s_all - s_pos
        y_norm = cnt * (np.float32(C) - cnt)
        total = total + np.sum(s_posinv * s_neg / y_norm, dtype=np.float32)
    return np.asarray(total / np.float32(B), dtype=np.float32)


if __name__ == "__main__":
    rng = np.random.default_rng(0)
    inp = rng.standard_normal((B, C), dtype=np.float32)
    tgt = rng.integers(0, 2, size=(B, C)).astype(np.int32)
    print(kernel(input=inp, target=tgt))
